# revision 43
# baseline (speedup 1.0000x reference)
"""DenseGCN (EdgeConv x4, dense concat, final group-max) on 8 TRN2 NeuronCores.

Algorithm (per EdgeConv block, weights w = [wa | wb], C = in-channels):
    msg_e = hi@wa.T + (hj-hi)@wb.T + b = hi@(wa-wb).T + hj@wb.T + b
so with p = h@(wa-wb).T (dst side) and q' = h@wb.T + b (src side):
    agg[n] = segment_max(msg)[n] = p[n] + max_{e: dst=n} q'[src_e]
Per-edge work is a 32-wide row gather + running max - no per-edge matmul.

Sharding: nodes (and their incoming edges) split 4096/core across 8 cores.
Per block each core computes its q' shard with small PE matmuls, an
AllGather assembles the full fp16 q' table (j-major rows padded to 256 B)
in each core's HBM, then dma_gather pulls the 65536 edge rows (2048-idx
chunks on 4 rotating SWDGE queues - the sweet spot for Q7 descriptor-gen
throughput, ~2.1 ns/idx) and DVE max-reduces over the K=16 edges per
node in fp16. agg is PE-transposed into the channel-major h needed by
the next block's matmuls; the final 5-way group-max folds per chunk.

Critical-path structure (HW-measured): each block's gather phase is HBM
random-read-bandwidth-bound: 65536 rows x 256 B = 16 MB/block/core at
~120 GB/s effective (2 cores share an HBM stack) = ~133 us/block. Q7
descriptor generation is NOT the bottleneck (true gen cost ~0.3 ns/idx;
the fat GpSimd slices are DMA-drain backpressure via the tile-pool WAR).
The 256 B minimum gather row (elem_size/stride % 256B asserts) forces 4x
read amplification over the 64 B payload - unavoidable with this ISA.
Between blocks, ~70-85 us is exposed: drain tail -> last reduce -> mm
tail -> bounce -> AllGather dispatch (~12 us) + mesh (~32 us for the
1 MB padded shard). Block 0's q' table is host-precomputed (it depends
only on x and weights), removing one AllGather entirely.

Measured dead ends (do not retry): prepare_only pregen (NaN, ucode
broken); single_packet=True (hangs the device); SBUF-source transposed
gather (correct but ~2.4 ns/idx - the 2 B-per-partition transpose
writes are slower than HBM reads); compact AG + strided expand (expand
costs 42 us serial); split AllGather, Shared (single-writer assert) or
Local (non-shared mesh ~2x slower, and the Pool-queue position needed
for early trigger stalls on the reduce lag); >4 SWDGE queues (ucode
max); gat pool bufs < ~10 (WRONG RESULTS - latent WAR race).
"""
import sys
import types

import numpy as np

if "/opt/trn_rl_repo" not in sys.path:
    sys.path.insert(0, "/opt/trn_rl_repo")


def _install_ntff_hook_shim():
    if "antenv.axon_hooks" in sys.modules:
        return
    try:
        import antenv
    except ImportError:
        return
    mod = types.ModuleType("antenv.axon_hooks")
    mod._hook = None
    mod.set_axon_ntff_profile_hook = lambda h: setattr(mod, "_hook", h)
    mod.get_axon_ntff_profile_hook = lambda: mod._hook
    sys.modules["antenv.axon_hooks"] = mod
    antenv.axon_hooks = mod
    try:
        from trn_agent_boot.trn_boot import _ntff_profile_via_ctypes

        hook = _ntff_profile_via_ctypes("/opt/axon/libaxon_pjrt.so")
        if hook is not None:
            mod._hook = hook
    except Exception:
        pass


_install_ntff_hook_shim()

import concourse.bacc as bacc
import concourse.mybir as mybir
import concourse.tile as tile
from concourse import bass_utils

N = 32768
GR = 32
NB = 4
NCORES = 8
NS = N // NCORES          # 4096 nodes per core
JT = 32                   # node tiles per core (128 nodes each, n_local = p*32 + j)
F32 = mybir.dt.float32
F16 = mybir.dt.float16
I16 = mybir.dt.int16
JA = JT                  # single full AllGather (split variants measured slower)
TAPER = 8                # last TAPER node tiles gather in two k-halves: the
                         # end-of-block DMA backlog (bufs-deep in-flight
                         # chunks) halves, so the last reduces + bounce write
                         # reach the AllGather sooner


def _chunk_plan(k_pad):
    """[(tile, k0, nk)] gather pieces per block; k-split tail when k_pad=16."""
    if k_pad != 16:
        return [(t, 0, k_pad) for t in range(JT)]
    plan = [(t, 0, 16) for t in range(JT - TAPER)]
    for t in range(JT - TAPER, JT):
        plan += [(t, 0, 8), (t, 8, 8)]
    return plan

_CACHE = {}


def _build_nc(k_pad: int):
    """Build + finalize the SPMD Bass program. k_pad = padded in-degree."""
    JW = 1                     # node-tile columns per gather chunk
    CH = JT // JW              # gather chunks per block (j-slices)
    CHUNK = JW * 128 * k_pad   # idxs per chunk (8192 for k_pad=16)
    GCOLS = CHUNK // 128       # gather groups per chunk (k*JW)
    ICOLS = CHUNK // 16        # idx columns per chunk
    NQ = 4                     # SWDGE queues (ucode max), rotated across gathers
    EL = 128                   # fp16 elements per gather row (256 B)

    nc = bacc.Bacc("TRN2", target_bir_lowering=False, debug=False,
                   enable_asserts=False, num_devices=NCORES,
                   num_swdge_queues=NQ, dynamic_dma_scratch_size=18432)

    xT = nc.dram_tensor("xT", [3, NS], F32, kind="ExternalInput")
    gidx = nc.dram_tensor("gidx", [128, CH * ICOLS], I16, kind="ExternalInput")
    eye = nc.dram_tensor("eye", [128, 128], F32, kind="ExternalInput")
    Ws, Bs = [], []
    for i in range(NB):
        cin = 3 if i == 0 else GR * (i + 1)
        wcols = 64
        Ws.append(nc.dram_tensor(f"W{i}", [cin, wcols], F32, kind="ExternalInput"))
        Bs.append(nc.dram_tensor(f"B{i}", [128, 8 * wcols], F32, kind="ExternalInput"))
    out = nc.dram_tensor("out", [NS, GR], F32, kind="ExternalOutput")

    qf0 = nc.dram_tensor("qfull0", [N, 128], F16, kind="ExternalInput")
    assert JW == 1 and CH == JT, "pipelined emission assumes JW=1 (k_pad=16)"

    with tile.TileContext(nc) as tc:
        with (
            tc.tile_pool(name="persist", bufs=1) as pp,
            tc.tile_pool(name="gat", bufs=12) as gp,
            tc.tile_pool(name="gath", bufs=8) as gph,
            tc.tile_pool(name="mhalf", bufs=2) as mh,
            tc.tile_pool(name="mtr", bufs=6) as mp,
            tc.tile_pool(name="psmm", bufs=2, space="PSUM") as psmm,
            tc.tile_pool(name="pstr", bufs=2, space="PSUM") as pstr,
            tc.tile_pool(name="dram", bufs=1, space="DRAM") as dp,
        ):
            # ---- persistent tiles -------------------------------------
            # gidx first, in two pieces: chunk 0's gather only waits on the
            # small first piece (subtile deps), shaving boot latency
            gidx_sb = pp.tile([128, CH * ICOLS], I16, tag="gidx")
            nc.sync.dma_start(gidx_sb[:, 0:4 * ICOLS], gidx[:, 0:4 * ICOLS])
            nc.sync.dma_start(gidx_sb[:, 4 * ICOLS:], gidx[:, 4 * ICOLS:])
            xT_sb = pp.tile([3, NS], F32, tag="xT")
            nc.sync.dma_start(xT_sb[:], xT[:, :])
            eye_sb = pp.tile([128, 128], F32, tag="eye")
            nc.sync.dma_start(eye_sb[:], eye[:, :])
            W_sb, B_sb = [], []
            for i in range(NB):
                cin = 3 if i == 0 else GR * (i + 1)
                wcols = 64
                w_t = pp.tile([cin, wcols], F32, tag=f"W{i}")
                nc.sync.dma_start(w_t[:], Ws[i][:, :])
                b_t = pp.tile([128, 8, wcols], F32, tag=f"B{i}")
                nc.sync.dma_start(b_t[:], Bs[i][:, :].rearrange("p (j c) -> p j c", j=8))
                W_sb.append(w_t)
                B_sb.append(b_t)
            # channel-major h (grows 32 rows per block; block-3 agg not needed here)
            hT = pp.tile([128, NS], F32, tag="hT")
            # node-major concat h: [p, j, col] with col = part*GR + c
            h_nm = pp.tile([128, JT, (NB + 1) * GR], F32, tag="h_nm")
            p_nm = pp.tile([128, JT, GR], F32, tag="p_nm")
            qpad = pp.tile([128, JT, EL], F16, tag="qpad")
            m_acc = pp.tile([128, JT, GR], F32, tag="m_acc")
            res_t = pp.tile([128, JT, GR], F32, tag="res")

            qfull = [qf0] + [
                dp.tile([N, EL], F16, tag=f"qfull{i}", name=f"qfull{i}",
                        addr_space="Shared")
                for i in range(1, NB)]
            bounce = [None] + [
                dp.tile([NS, EL], F16, tag=f"bounce{i}", name=f"bounce{i}")
                for i in range(1, NB)]

            # ---- emit helpers -----------------------------------------
            # The program is software-pipelined at EMISSION level: the Pool
            # engine executes its queue in order, so block (i+1)'s AllGather
            # trigger must sit BETWEEN block i's gather instructions to fire
            # mid-phase. mm(i+1) groups are likewise emitted inside block i's
            # chunk loop (right after the reduces they consume) so the bounce
            # streams early and AG-A (rows j<JA) is ready when Pool reaches
            # its trigger at chunk AGA_POS. Only AG-B (last (32-JA)/32 of the
            # shard) stays exposed between blocks.
            def emit_mm_group(i, g, grp=2, wcols=64):
                cin = 3 if i == 0 else GR * (i + 1)
                lhs_src = xT_sb if i == 0 else hT
                with nc.named_scope(f"mm{i}"):
                    ps = psmm.tile([128, grp, wcols], F32, tag="mmps",
                                   name="ps")
                    for jj in range(grp):
                        j = g * grp + jj
                        lhsT = lhs_src[0:cin, :].rearrange(
                            "c (p j) -> c j p", p=128)[:, j, :]
                        nc.tensor.matmul(ps[:, jj, :], lhsT, W_sb[i][:],
                                         start=True, stop=True)
                    # PSUM -> SBUF with fused bias add (q' cast to fp16)
                    js = slice(g * grp, (g + 1) * grp)
                    bias = B_sb[i][:, 0:grp, :]
                    if i == 0:
                        # host precomputes the block-0 q' table; the mm
                        # only produces p and h0
                        nc.vector.tensor_add(p_nm[:, js, :],
                                             ps[:, :, 0:GR],
                                             bias[:, :, 0:GR])
                        nc.vector.tensor_add(h_nm[:, js, 0:GR],
                                             ps[:, :, GR:2 * GR],
                                             bias[:, :, GR:2 * GR])
                        return
                    nc.vector.tensor_add(qpad[:, js, 0:GR], ps[:, :, 0:GR],
                                         bias[:, :, 0:GR])
                    nc.vector.tensor_add(p_nm[:, js, :], ps[:, :, GR:2 * GR],
                                         bias[:, :, GR:2 * GR])
                    # stream this group's q' rows into the bounce
                    # (j-major rows: row = j*128 + p)
                    rh = slice(g * grp * 128, (g + 1) * grp * 128)
                    nc.sync.dma_start(
                        bounce[i][rh, :].rearrange("(j p) c -> p j c", j=grp),
                        qpad[:, js, :])

            def emit_ag(i):
                with nc.named_scope(f"ag{i}"):
                    nc.gpsimd.collective_compute(
                        "AllGather", mybir.AluOpType.bypass,
                        replica_groups=[list(range(NCORES))],
                        ins=[bounce[i].opt()], outs=[qfull[i].opt()])

            plan = _chunk_plan(k_pad)
            offs, off = [], 0
            for (_t, _k0, _nk) in plan:
                offs.append(off)
                off += _nk * 128 // 16
            assert off == CH * ICOLS

            def emit_gather(i, c):
                t, k0, nk = plan[c]
                pool = gp if nk == k_pad else gph
                g_t = pool.tile([128, nk * JW, EL], F16, tag="gather",
                                name="g_t")
                nc.gpsimd.dma_gather(
                    out_ap=g_t[:],
                    in_ap=qfull[i][:, :],
                    idxs_ap=gidx_sb[:, offs[c]:offs[c] + nk * 128 // 16],
                    num_idxs=nk * 128, num_idxs_reg=nk * 128,
                    elem_size=EL, single_packet=False,
                    queue_num=c % NQ)
                return g_t

            def emit_tree(gv, nk, dst):
                """halving max tree over gv [p, nk, jl, GR] -> dst [p,jl,GR]"""
                ve = nc.vector
                kk = nk
                if kk > 2:
                    st = mp.tile([128, k_pad // 2, JW, GR], F16,
                                 tag="mtree", name="st")
                    h = kk // 2
                    ve.tensor_max(st[:, 0:h], gv[:, 0:h], gv[:, h:kk])
                    kk = h
                    while kk > 2:
                        h = kk // 2
                        ve.tensor_max(st[:, 0:h], st[:, 0:h], st[:, h:kk])
                        kk = h
                    gv = st[:]
                if kk == 2:
                    ve.tensor_max(dst, gv[:, 0], gv[:, 1])
                else:
                    ve.tensor_copy(dst, gv[:, 0])

            def emit_reduce(i, c, g_t):
                t, k0, nk = plan[c]
                c0 = (i + 1) * GR
                js = slice(t * JW, (t + 1) * JW)
                ve = nc.vector
                gv = g_t[:].rearrange(
                    "p (k j) c -> p k j c", k=nk)[:, :, :, 0:GR]
                if k0 == 0:
                    emit_tree(gv, nk, m_acc[:, js, :])
                else:
                    mh_t = mh.tile([128, JW, GR], F16, tag="mhalf",
                                   name="mh_t")
                    emit_tree(gv, nk, mh_t[:])
                    ve.tensor_max(m_acc[:, js, :], m_acc[:, js, :], mh_t[:])
                if k0 + nk < k_pad:
                    return False        # tile t not complete yet
                ve.tensor_add(h_nm[:, js, c0:c0 + GR],
                              p_nm[:, js, :], m_acc[:, js, :])
                a = t
                if i < NB - 1:
                    # transpose agg (and h0 for block 0) into hT rows
                    grps = [(i + 1, GR * (i + 1))]
                    if i == 0:
                        grps.append((0, 0))
                    for src_grp, row0 in grps:
                        pt = pstr.tile([GR, JW, 128], F32, tag="trps",
                                       name="pt")
                        for jl in range(JW):
                            nc.tensor.transpose(
                                pt[:, jl, :],
                                h_nm[:, a * JW + jl,
                                     src_grp * GR:(src_grp + 1) * GR],
                                eye_sb[:])
                        dst = hT[row0:row0 + GR, :].rearrange(
                            "c (p j) -> c j p", p=128)[:, js, :]
                        nc.vector.tensor_copy(dst, pt[:])
                else:
                    # last block: fold the 5-way group-max per chunk
                    # res[n, f] = max_v hcat[n, 5f + v], then stream
                    # this chunk's rows straight out (no final tail)
                    hv = h_nm[:, js, :].rearrange(
                        "p j (f v) -> p j f v", v=NB + 1)
                    nc.vector.tensor_max(res_t[:, js, :],
                                         hv[:, :, :, 0], hv[:, :, :, 1])
                    nc.vector.tensor_max(res_t[:, js, :],
                                         res_t[:, js, :], hv[:, :, :, 2])
                    nc.vector.tensor_max(res_t[:, js, :],
                                         res_t[:, js, :], hv[:, :, :, 3])
                    nc.vector.tensor_max(res_t[:, js, :],
                                         res_t[:, js, :], hv[:, :, :, 4])
                    with nc.named_scope("final"):
                        nc.sync.dma_start(
                            out[:, :].rearrange(
                                "(p j) c -> p j c", p=128)[:, js, :],
                            res_t[:, js, :])
                return True

            # ---- pipelined program ------------------------------------
            for g in range(JT // 2):
                emit_mm_group(0, g)
            for i in range(NB):
                with nc.named_scope(f"gather{i}"):
                    for c in range(len(plan)):
                        g_t = emit_gather(i, c)
                        done = emit_reduce(i, c, g_t)
                        t = plan[c][0]
                        if done and i < NB - 1 and t % 2 == 1:
                            emit_mm_group(i + 1, (t - 1) // 2)
                if i < NB - 1:
                    emit_ag(i + 1)

    nc.finalize()
    return nc


def _prep_host(x, edge_index, lin_x_w, lin_x_b, ws, bs):
    """Host-side sharding + weight folding. Returns (k_pad, in_maps)."""
    src = np.asarray(edge_index[0], dtype=np.int64)
    dst = np.asarray(edge_index[1], dtype=np.int64)
    E = src.shape[0]

    # per-dst source lists S[k, n]
    if E % N == 0 and np.array_equal(dst, np.tile(np.arange(N, dtype=dst.dtype),
                                                  E // N)):
        S = src.reshape(E // N, N)
    else:
        counts = np.bincount(dst, minlength=N)
        k_max = max(int(counts.max()), 1)
        S = np.empty((k_max, N), dtype=np.int64)
        order = np.argsort(dst, kind="stable")
        ssrc, sdst = src[order], dst[order]
        starts = np.zeros(N + 1, dtype=np.int64)
        np.cumsum(counts, out=starts[1:])
        for n in range(N):
            c = counts[n]
            seg = ssrc[starts[n]:starts[n + 1]]
            if c == 0:
                S[:, n] = n  # no incoming edges: arbitrary (ref gives -inf)
            else:
                S[:c, n] = seg
                S[c:, n] = seg[0]  # duplicate an edge - max unchanged
    k_pad = S.shape[0]
    if k_pad & (k_pad - 1):  # pad to power of two (duplicates keep max exact)
        tgt = 1 << (k_pad - 1).bit_length()
        S = np.concatenate([S] + [S[:1]] * (tgt - k_pad), axis=0)
        k_pad = tgt

    # folded weights: W_i = [wb.T | (wa-wb).T (| lxw.T)] with bias on q
    x = np.asarray(x, dtype=np.float32)
    lxw = np.asarray(lin_x_w, dtype=np.float64)
    lxb = np.asarray(lin_x_b, dtype=np.float64)
    W_list, B_list = [], []
    for i in range(NB):
        w = np.asarray(ws[i], dtype=np.float64)
        b = np.asarray(bs[i], dtype=np.float64)
        C = GR * (i + 1)
        wa, wb = w[:, :C], w[:, C:]
        wd = wa - wb
        if i == 0:
            W = np.concatenate([(wd @ lxw).T, lxw.T], axis=1)
            B = np.concatenate([wd @ lxb, lxb])
        else:
            W = np.concatenate([wb.T, wd.T], axis=1)
            B = np.concatenate([b, np.zeros(GR)])
        W_list.append(np.ascontiguousarray(W, dtype=np.float32))
        B_list.append(np.ascontiguousarray(
            np.tile(B[None, :], (128, 8)), dtype=np.float32))

    eye = np.eye(128, dtype=np.float32)
    plan = _chunk_plan(k_pad)

    # remap source node ids to j-major qfull table rows (the bounce is
    # written j-major so each mm group's rows are contiguous). The table is
    # assembled by TWO partial AllGathers (j<JA | j>=JA), each concatenating
    # 8 partial shards.
    r_, nl_ = np.divmod(S, NS)
    p_, j_ = np.divmod(nl_, JT)
    la = JA * 128
    ell_ = j_ * 128 + p_
    S = np.where(j_ < JA, r_ * la + ell_,
                 NCORES * la + r_ * (JT * 128 - la) + (ell_ - la))

    # host-computed block-0 q' table (q'_0 = h0 @ wb0.T + b0), laid out in
    # the j-major padded-row format the gathers expect
    w0 = np.asarray(ws[0], dtype=np.float64)
    wb0 = w0[:, GR:]
    h0 = x.astype(np.float64) @ lxw.T + lxb
    q0 = (h0 @ wb0.T + np.asarray(bs[0], dtype=np.float64)).astype(np.float16)
    nn = np.arange(N)
    r0, nl0 = np.divmod(nn, NS)
    p0, j0 = np.divmod(nl0, JT)
    ell0 = j0 * 128 + p0
    rows = np.where(j0 < JA, r0 * la + ell0,
                    NCORES * la + r0 * (JT * 128 - la) + (ell0 - la))
    Q0 = np.zeros((N, 128), dtype=np.float16)
    Q0[rows, 0:GR] = q0
    in_maps = []
    for r in range(NCORES):
        xs = np.ascontiguousarray(x[r * NS:(r + 1) * NS].T)
        # gather idx per _chunk_plan piece (t, k0, nk): seq position
        #   e = k_local*128 + p -> S[k0+k_local, r*NS + p*32 + t];
        #   wrapped 16-partition layout, replicated x8
        Sr = S[:, r * NS:(r + 1) * NS].reshape(k_pad, 128, JT)  # [k, p, t]
        segs = []
        for (t, k0, nk) in plan:
            seq = Sr[k0:k0 + nk, :, t].reshape(-1)
            segs.append(seq.reshape(-1, 16).T)                  # [16, nk*8]
        cols = np.concatenate(segs, axis=1)
        gidx_np = np.ascontiguousarray(np.tile(cols, (8, 1)), dtype=np.int16)
        im = {"xT": xs, "gidx": gidx_np, "eye": eye, "qfull0": Q0}
        for i in range(NB):
            im[f"W{i}"] = W_list[i]
            im[f"B{i}"] = B_list[i]
        in_maps.append(im)
    return k_pad, in_maps


def kernel(x, edge_index, lin_x_w, lin_x_b, w0, b0, w1, b1, w2, b2, w3, b3,
           _trace=False):
    k_pad, in_maps = _prep_host(x, edge_index, lin_x_w, lin_x_b,
                                (w0, w1, w2, w3), (b0, b1, b2, b3))
    if k_pad not in _CACHE:
        _CACHE[k_pad] = _build_nc(k_pad)
    nc = _CACHE[k_pad]
    res = bass_utils.run_bass_kernel_spmd(
        nc, in_maps, core_ids=list(range(NCORES)), trace=_trace)
    full = np.concatenate([res.results[r]["out"] for r in range(NCORES)], axis=0)
    if _trace:
        kernel.last_results = res
    return full

